# revision 1
# baseline (speedup 1.0000x reference)
"""LongNet dilated-attention transformer block on 8 Trainium2 NeuronCores.

All three branches use rate=2 with even segment sizes, so they all read
the same subsampled tokens x[:, ::2, :]; they differ only in attention
block size m in {256, 512, 1024}. Shard the (B*4096) even tokens into 8
shards of 2048 (multiple of the largest block) -> pure data parallelism
across the 8 cores, weights broadcast, no collectives.
"""

import numpy as np
from functools import partial

import jax
import jax.numpy as jnp

D = 1024
H = 16
HD = D // H
SEGS = (512, 1024, 2048)
BLOCKS = tuple(s // 2 for s in SEGS)  # (256, 512, 1024)
B, S = 4, 8192
SCALE = 1.0 / np.float32(np.sqrt(HD))
N_CORES = 8
T = (B * S // 2) // N_CORES  # 2048 tokens per core

BF = jnp.bfloat16
F32 = jnp.float32


def _branch(x, xc, m, qkv_wT, qkv_b, out_wT, out_b):
    """x fp32 / xc bf16: (T, D); block-diagonal attention with block m."""
    n = T // m
    # projection in bf16 inputs, fp32 accumulation (4x PE rate vs fp32)
    qkv = jnp.matmul(xc, qkv_wT, preferred_element_type=F32) + qkv_b
    q, k, v = jnp.split(qkv, 3, axis=-1)
    q = q.reshape(n, m, H, HD)
    k = k.reshape(n, m, H, HD)
    v = v.reshape(n, m, H, HD)
    s = jnp.einsum('nqhd,nkhd->nhqk', q, k) * SCALE
    # scores are O(1) here; skip max-subtraction (exact same softmax value)
    e = jnp.exp(s)
    a = e / e.sum(axis=-1, keepdims=True)
    o = jnp.einsum('nhqk,nkhd->nqhd', a, v).reshape(T, D)
    return jnp.matmul(o.astype(BF), out_wT, preferred_element_type=F32) + out_b


@partial(jax.pmap, in_axes=0)
def _shard_fn(x, qw0, qb0, ow0, ob0, qw1, qb1, ow1, ob1, qw2, qb2, ow2, ob2):
    params = ((qw0, qb0, ow0, ob0), (qw1, qb1, ow1, ob1), (qw2, qb2, ow2, ob2))
    xc = x.astype(BF)
    out = None
    for m, (qw, qb, ow, ob) in zip(BLOCKS, params):
        y = _branch(x, xc, m, qw.T.astype(BF), qb, ow.T.astype(BF), ob)
        out = y if out is None else out + y
    return out


def kernel(x, qkv_w0, qkv_b0, out_w0, out_b0,
           qkv_w1, qkv_b1, out_w1, out_b1,
           qkv_w2, qkv_b2, out_w2, out_b2):
    x = np.asarray(x)
    xe = np.ascontiguousarray(x[:, ::2, :]).reshape(N_CORES, T, D)
    devs = jax.devices()[:N_CORES]
    args = [jax.device_put_replicated(np.asarray(a), devs) for a in
            (qkv_w0, qkv_b0, out_w0, out_b0,
             qkv_w1, qkv_b1, out_w1, out_b1,
             qkv_w2, qkv_b2, out_w2, out_b2)]
    xs = jax.device_put_sharded(list(xe), devs)
    y = _shard_fn(xs, *args)
    y = np.asarray(jax.device_get(y))              # (8, 2048, D)
    return y.reshape(B, S // 2, D)

